# revision 11
# baseline (speedup 1.0000x reference)
"""Trainium2 Bass kernel for nn_Attention2 (B=4, N=4096, W=1024, H=16, A=64).

Sharding: 8 cores = batch(4) x head-half(2). Each core computes the partial
output sum over its 8 heads for one batch; the host adds the two half-sums.

Math (per batch b, head h), with c = exp(x@k1 + p1), e = exp(x@(k2-k3) - p2)
[= diag/(extra*p2e)], g = e/c = exp(x@(k2-k3-k1) - p2 - p1):
    C[t] = cumsum(c);  den = C + e;  rden = 1/den;  w = c*rden
    ratio'[t] = den[t-1]*rden[t]
    z[t] = ratio'[t]*z[t-1] + v[t]*w[t]      (v = x@vw;  z = cumsum(c*v)/den)
    out = z + (v*w)*g;  y = sum_h out @ owT
p1/p2 (sums of 64 near-linear sinusoids) are expanded as cubic polynomials in
n and folded into the k-projection matmul via 4 extra contraction rows of
[1, n, n^2, n^3].

The chunk loop is software-pipelined: PE runs rows_ci, then values_ci
interleaved with the three per-pair broadcasts of chunk ci, then the output
matmuls of chunk ci-1 — so the elementwise backend (ACT/DVE/Pool) of chunk ci
overlaps with PE work and PE never idles.
"""

import numpy as np

import concourse.bacc as bacc
import concourse.mybir as mybir
import concourse.tile as tile
from concourse.bass_utils import run_bass_kernel_spmd

F32 = mybir.dt.float32
F32R = mybir.dt.float32r
AF = mybir.ActivationFunctionType
OP = mybir.AluOpType

B, N, W, H, A, P = 4, 4096, 1024, 16, 64, 64
HL = 8            # heads per core
NPAIR = 4         # head pairs per core
CHUNK = 512
NCHUNK = N // CHUNK          # 8
KB = W // 128                # 8 x-K-blocks
NBLK = CHUNK // 128          # n-blocks per chunk for stage-3

_NC_CACHE = {}


def _build(reps=1, mode="base"):
    key = ("nc", reps, mode)
    if key in _NC_CACHE:
        return _NC_CACHE[key]
    nc = bacc.Bacc("TRN2")

    xtb = nc.dram_tensor("xtb", [W + 4, N], F32R, kind="ExternalInput")
    kpack = nc.dram_tensor("kpack", [W + 4, 72], F32R, kind="ExternalInput")
    vwp = nc.dram_tensor("vwp", [NPAIR, W, 128], F32R, kind="ExternalInput")
    owtp = nc.dram_tensor("owtp", [NPAIR, 128, W], F32R, kind="ExternalInput")
    selp = nc.dram_tensor("selp", [NPAIR, 8, 128], F32R, kind="ExternalInput")
    y = nc.dram_tensor("y", [N, W], F32, kind="ExternalOutput")

    with tile.TileContext(nc) as tc:
        import os
        _bufs = dict(x.split("=") for x in os.environ.get("KBUFS", "").split(",") if x)
        bf = lambda k, d: int(_bufs.get(k, d))
        with (
            tc.tile_pool(name="const", bufs=1) as const,
            tc.tile_pool(name="xtp", bufs=bf("xt", 2)) as xtp,
            tc.tile_pool(name="rowp", bufs=bf("row", 2)) as rowp,
            tc.tile_pool(name="bigp", bufs=bf("big", 2)) as bigp,
            tc.tile_pool(name="innp", bufs=bf("inn", 8)) as innp,
            tc.tile_pool(name="yp", bufs=bf("yp", 3)) as yp,
            tc.tile_pool(name="rows_ps", bufs=bf("rps", 1), space="PSUM") as rows_ps,
            tc.tile_pool(name="v_ps", bufs=bf("vps", 2), space="PSUM") as v_ps,
            tc.tile_pool(name="bc_ps", bufs=bf("bcps", 1), space="PSUM") as bc_ps,
            tc.tile_pool(name="y_ps", bufs=bf("yps", 2), space="PSUM") as y_ps,
        ):
            # ---- resident weights, ordered by first use so PE starts ASAP:
            # kp8/kp4 + chunk-0 xt + basis first, then vw, sel; owt is
            # deferred until after the first xt prefetch (needed ~30us in).
            kp8 = const.tile([128, KB, 72], F32R, name="kp8", tag="kp8")
            nc.sync.dma_start(
                out=kp8,
                in_=kpack[0:W, :].rearrange("(kb p) m -> p kb m", p=128))
            kp4 = const.tile([4, 72], F32R, name="kpbas", tag="kpbas")
            nc.sync.dma_start(out=kp4, in_=kpack[W:W + 4, :])

            def load_xt(ci):
                c0 = ci * CHUNK
                ts = []
                for kb in range(KB):
                    t = xtp.tile([128, CHUNK], F32R, name=f"xt{kb}",
                                 tag=f"xt{kb}")
                    nc.sync.dma_start(
                        out=t, in_=xtb[kb * 128:(kb + 1) * 128, c0:c0 + CHUNK])
                    ts.append(t)
                bas = xtp.tile([4, CHUNK], F32R, name="bas", tag="bas")
                nc.sync.dma_start(out=bas, in_=xtb[W:W + 4, c0:c0 + CHUNK])
                return ts, bas

            xt_cur = load_xt(0)

            # weights on the ACT HW-DGE ring, parallel with xt0 on SP
            vw_sb = []
            for p in range(NPAIR):
                t = const.tile([128, KB, 128], F32R, name=f"vw{p}", tag=f"vw{p}")
                nc.scalar.dma_start(
                    out=t,
                    in_=vwp[p].rearrange("(kb q) m -> q kb m", q=128))
                vw_sb.append([t[:, kb, :] for kb in range(KB)])

            sel8 = const.tile([8, NPAIR, 128], F32R, name="sel8", tag="sel8")
            nc.scalar.dma_start(out=sel8, in_=selp.rearrange("p r m -> r p m"))
            sel_sb = [sel8[:, p, :] for p in range(NPAIR)]

            owt_sb = []

            def load_owt():
                for p in range(NPAIR):
                    t = const.tile([128, W], F32R, name=f"owt{p}",
                                   tag=f"owt{p}")
                    nc.scalar.dma_start(out=t, in_=owtp[p, :, :])
                    owt_sb.append(t)

            ones8 = const.tile([HL, CHUNK], F32)
            nc.vector.memset(ones8, 1.0)

            # ---- per-chunk pipeline state ----
            z_prev = [None] * NPAIR     # z carry tiles per pair
            c_prev = None               # C-scan carry tile
            den_prev = None             # previous chunk's den tile
            pend = None                 # (inner tiles, c0) awaiting stage-3

            total = NCHUNK * reps
            cis = [i % NCHUNK for i in range(total)]

            def emit_back(inner, c0):
                """Stage 3 for a completed chunk: y matmuls + copies + DMA."""
                for nb in range(NBLK):
                    for wh in range(2):
                        yps = y_ps.tile([128, 512], F32, tag="y")
                        for p in range(NPAIR):
                            nc.tensor.matmul(
                                yps,
                                lhsT=inner[p][:, nb * 128:(nb + 1) * 128],
                                rhs=owt_sb[p][:, wh * 512:(wh + 1) * 512],
                                start=(p == 0), stop=(p == NPAIR - 1))
                        y_sb = yp.tile([128, 512], F32, tag="y_sb")
                        nc.scalar.copy(y_sb, yps)
                        nc.sync.dma_start(
                            out=y[c0 + nb * 128:c0 + (nb + 1) * 128,
                                  wh * 512:(wh + 1) * 512],
                            in_=y_sb)

            for it in range(total):
                ci = cis[it]
                c0 = ci * CHUNK
                first = (it == 0)

                xt, bas = xt_cur
                if it + 1 < total:
                    xt_next = load_xt(cis[it + 1])
                else:
                    xt_next = None

                # ---- row projections: [24, CHUNK] psum ----
                rows = rows_ps.tile([72, CHUNK], F32, tag="rows")
                for kb in range(KB):
                    nc.tensor.matmul(rows, lhsT=kp8[:, kb, :], rhs=xt[kb],
                                     start=(kb == 0), stop=False)
                nc.tensor.matmul(rows, lhsT=kp4, rhs=bas,
                                 start=False, stop=True)

                # exps: c, e, g  [8, CHUNK] each
                c_t = rowp.tile([HL, CHUNK], F32R, tag="c_t")
                nc.scalar.activation(c_t, rows[0:8, :], AF.Exp)
                e_t = rowp.tile([HL, CHUNK], F32R, tag="e_t")
                nc.scalar.activation(e_t, rows[32:40, :], AF.Exp)
                g_t = rowp.tile([HL, CHUNK], F32R, tag="g_t")
                nc.scalar.activation(g_t, rows[64:72, :], AF.Exp)

                # C = cumsum(c) chunk-chained (Pool)
                c_ch = rowp.tile([HL, CHUNK], F32, tag="c_ch")
                nc.vector.tensor_tensor_scan(
                    c_ch, data0=ones8, data1=c_t,
                    initial=(0.0 if c_prev is None
                             else c_prev[:, CHUNK - 1:CHUNK]),
                    op0=OP.mult, op1=OP.add)
                c_prev = c_ch
                # den = C + e ; rden = 1/den ; w = c*rden (DVE)
                den = rowp.tile([HL, CHUNK], F32, tag="den")
                nc.vector.tensor_add(den, c_ch, e_t)
                rden = rowp.tile([HL, CHUNK], F32, tag="rden")
                nc.vector.reciprocal_approx_fast(out=rden, in_=den)
                w_t = rowp.tile([HL, CHUNK], F32R, tag="w_t")
                nc.vector.tensor_mul(w_t, c_t, rden)
                # ratio'[t] = den[t-1] * rden[t]
                rat = rowp.tile([HL, CHUNK], F32R, tag="rat")
                nc.vector.tensor_mul(rat[:, 1:CHUNK], den[:, 0:CHUNK - 1],
                                     rden[:, 1:CHUNK])
                if den_prev is None:
                    # any finite value works: initial z state is 0
                    nc.vector.tensor_copy(rat[:, 0:1], ones8[:, 0:1])
                else:
                    nc.vector.tensor_mul(rat[:, 0:1],
                                         den_prev[:, CHUNK - 1:CHUNK],
                                         rden[:, 0:1])
                den_prev = den

                # ---- values + broadcasts, interleaved on PE ----
                vps_l = [None] * NPAIR
                bc_l = [None] * NPAIR

                def emit_values(p):
                    vps = v_ps.tile([128, CHUNK], F32, tag="v")
                    for kb in range(KB):
                        nc.tensor.matmul(vps, lhsT=vw_sb[p][kb], rhs=xt[kb],
                                         start=(kb == 0), stop=(kb == KB - 1))
                    vps_l[p] = vps
                    v_sb = bigp.tile([128, CHUNK], F32, tag="v_sb")
                    nc.scalar.copy(v_sb, vps)
                    return v_sb

                def emit_bcast(p):
                    r_rep = bc_ps.tile([128, CHUNK], F32, tag="r_rep")
                    nc.tensor.matmul(r_rep, lhsT=sel_sb[p], rhs=rat,
                                     start=True, stop=True)
                    w_rep = bc_ps.tile([128, CHUNK], F32, tag="w_rep")
                    nc.tensor.matmul(w_rep, lhsT=sel_sb[p], rhs=w_t,
                                     start=True, stop=True)
                    g_rep = bc_ps.tile([128, CHUNK], F32, tag="g_rep")
                    nc.tensor.matmul(g_rep, lhsT=sel_sb[p], rhs=g_t,
                                     start=True, stop=True)
                    w_sb = bigp.tile([128, CHUNK], F32, tag="w_sb")
                    nc.vector.tensor_copy(w_sb, w_rep)
                    bc_l[p] = (r_rep, g_rep, w_sb)

                v_sb_l = [None] * NPAIR
                # PE order: v0 v1 v2 bc0 v3 bc1 bc2 bc3
                v_sb_l[0] = emit_values(0)
                v_sb_l[1] = emit_values(1)
                v_sb_l[2] = emit_values(2)
                emit_bcast(0)
                v_sb_l[3] = emit_values(3)
                emit_bcast(1)
                emit_bcast(2)
                emit_bcast(3)

                # ---- backend: vw, scan, t2, inner per pair ----
                inner = []
                for p in range(NPAIR):
                    r_rep, g_rep, w_sb = bc_l[p]
                    vw = bigp.tile([128, CHUNK], F32, tag="vw", bufs=4)
                    nc.gpsimd.tensor_mul(vw, v_sb_l[p], w_sb)
                    z_sb = bigp.tile([128, CHUNK], F32, tag="z_sb", bufs=8)
                    nc.vector.tensor_tensor_scan(
                        z_sb, data0=r_rep, data1=vw,
                        initial=(0.0 if z_prev[p] is None
                                 else z_prev[p][:, CHUNK - 1:CHUNK]),
                        op0=OP.mult, op1=OP.add)
                    z_prev[p] = z_sb
                    t2 = bigp.tile([128, CHUNK], F32, tag="t2")
                    nc.vector.tensor_mul(t2, vw, g_rep)
                    inn = innp.tile([128, CHUNK], F32R, name="inner",
                                    tag="inner")
                    nc.gpsimd.tensor_add(inn, z_sb, t2)
                    inner.append(inn)

                if it == 0:
                    load_owt()

                # ---- lagged stage 3 ----
                if pend is not None:
                    emit_back(*pend)
                pend = (inner, c0)
                xt_cur = xt_next

            emit_back(*pend)

    nc.finalize()
    _NC_CACHE[key] = nc
    return nc


def _host_prep(x, k1, k2, k3, a1, a2, b1, b2, c, value_weight, output_weight):
    """Build the 8 per-core input maps."""
    x = np.asarray(x, np.float32)
    k1 = np.asarray(k1, np.float32)
    k2 = np.asarray(k2, np.float32)
    k3 = np.asarray(k3, np.float32)
    a1 = np.asarray(a1, np.float64)[..., 0]   # [H, P]
    a2 = np.asarray(a2, np.float64)[..., 0]
    b1 = np.asarray(b1, np.float64)[..., 0]
    b2 = np.asarray(b2, np.float64)[..., 0]
    cc = np.asarray(c, np.float64)[..., 0]
    vw = np.asarray(value_weight, np.float32)   # [H, W, A]
    ow = np.asarray(output_weight, np.float32)  # [H, W, A]

    n = np.linspace(0.0, 1.0, N)
    basis = np.stack([np.ones_like(n), n, n * n, n ** 3]).astype(np.float32)

    def taylor(a, b):
        # coef[k, h] of n^k for sum_p c*sin(a*n+b)
        s, co = np.sin(b), np.cos(b)
        c0 = (cc * s).sum(1)
        c1 = (cc * a * co).sum(1)
        c2 = -(cc * a * a * s).sum(1) / 2.0
        c3 = -(cc * a ** 3 * co).sum(1) / 6.0
        return np.stack([c0, c1, c2, c3])      # [4, H]

    p1c = taylor(a1, b1)
    p2c = taylor(a2, b2)

    xt_by_b = [np.empty((W + 4, N), np.float32) for _ in range(B)]
    for b in range(B):
        xt_by_b[b][:W] = x[b].T
        xt_by_b[b][W:] = basis

    selp = np.zeros((NPAIR, 8, 128), np.float32)
    for p in range(NPAIR):
        selp[p, 2 * p, 0:64] = 1.0
        selp[p, 2 * p + 1, 64:128] = 1.0

    in_maps = []
    for core in range(8):
        b, half = divmod(core, 2)
        hs = slice(half * HL, (half + 1) * HL)
        kpk = np.zeros((W + 4, 72), np.float32)
        # zc -> c = exp(x@k1 + p1)  (row groups 32-aligned: PSUM reads
        # by the ACT engine require 32-aligned partition bases)
        kpk[:W, 0:8] = k1[hs].T
        kpk[W:, 0:8] = p1c[:, hs]
        # ze -> e = exp(x@(k2-k3) - p2)
        kpk[:W, 32:40] = (k2[hs] - k3[hs]).T
        kpk[W:, 32:40] = -p2c[:, hs]
        # zg -> g = e/c = exp(x@(k2-k3-k1) - p2 - p1)
        kpk[:W, 64:72] = (k2[hs] - k3[hs] - k1[hs]).T
        kpk[W:, 64:72] = -(p2c[:, hs] + p1c[:, hs])

        vwp = np.empty((NPAIR, W, 128), np.float32)
        owtp = np.empty((NPAIR, 128, W), np.float32)
        for p in range(NPAIR):
            h0 = half * HL + 2 * p
            vwp[p, :, 0:64] = vw[h0]
            vwp[p, :, 64:128] = vw[h0 + 1]
            owtp[p, 0:64, :] = ow[h0].T
            owtp[p, 64:128, :] = ow[h0 + 1].T

        in_maps.append(dict(xtb=xt_by_b[b], kpack=kpk, vwp=vwp, owtp=owtp,
                            selp=selp))
    return in_maps


LAST_RESULT = None


def kernel(**inputs) -> np.ndarray:
    global LAST_RESULT
    in_maps = _host_prep(**inputs)
    nc = _build()
    res = None
    for attempt in range(3):
        try:
            res = run_bass_kernel_spmd(nc, in_maps, core_ids=list(range(8)))
            break
        except Exception:
            # transient axon-tunnel / device flakes happen; retry
            if attempt == 2:
                raise
            import time
            time.sleep(5)
    LAST_RESULT = res
    out = np.empty((B, N, W), np.float32)
    for b in range(B):
        out[b] = res.results[2 * b]["y"] + res.results[2 * b + 1]["y"]
    return out


# revision 13
# speedup vs baseline: 1.2374x; 1.2374x over previous
"""Trainium2 Bass kernel for nn_Attention2 (B=4, N=4096, W=1024, H=16, A=64).

Sharding: 8 cores = batch(4) x head-half(2). Each core computes the partial
output sum over its 8 heads for one batch; the host adds the two half-sums.

Math (per batch b, head h), with c = exp(x@k1 + p1), e = exp(x@(k2-k3) - p2)
[= diag/(extra*p2e)], g = e/c = exp(x@(k2-k3-k1) - p2 - p1):
    C[t] = cumsum(c);  den = C + e;  rden = 1/den;  w = c*rden
    ratio'[t] = den[t-1]*rden[t]
    z[t] = ratio'[t]*z[t-1] + v[t]*w[t]      (v = x@vw;  z = cumsum(c*v)/den)
    out = z + (v*w)*g;  y = sum_h out @ owT
p1/p2 (sums of 64 near-linear sinusoids) are expanded as cubic polynomials in
n and folded into the k-projection matmul via 4 extra contraction rows of
[1, n, n^2, n^3].

The chunk loop is software-pipelined: PE runs rows_ci, then values_ci
interleaved with the three per-pair broadcasts of chunk ci, then the output
matmuls of chunk ci-1 — so the elementwise backend (ACT/DVE/Pool) of chunk ci
overlaps with PE work and PE never idles.
"""

import numpy as np

import concourse.bacc as bacc
import concourse.mybir as mybir
import concourse.tile as tile
from concourse.bass_utils import run_bass_kernel_spmd

F32 = mybir.dt.float32
F32R = mybir.dt.float32r
AF = mybir.ActivationFunctionType
OP = mybir.AluOpType

B, N, W, H, A, P = 4, 4096, 1024, 16, 64, 64
HL = 8            # heads per core
NPAIR = 4         # head pairs per core
CHUNK = 512
NCHUNK = N // CHUNK          # 8
KB = W // 128                # 8 x-K-blocks
NBLK = CHUNK // 128          # n-blocks per chunk for stage-3

_NC_CACHE = {}


def _build(reps=1, mode="base"):
    key = ("nc", reps, mode)
    if key in _NC_CACHE:
        return _NC_CACHE[key]
    nc = bacc.Bacc("TRN2")

    xtb = nc.dram_tensor("xtb", [W + 4, N], F32R, kind="ExternalInput")
    kpack = nc.dram_tensor("kpack", [W + 4, 72], F32R, kind="ExternalInput")
    vwp = nc.dram_tensor("vwp", [NPAIR, W, 128], F32R, kind="ExternalInput")
    owtp = nc.dram_tensor("owtp", [NPAIR, 128, W], F32R, kind="ExternalInput")
    selp = nc.dram_tensor("selp", [NPAIR, 72, 128], F32R, kind="ExternalInput")
    y = nc.dram_tensor("y", [N, W], F32, kind="ExternalOutput")

    with tile.TileContext(nc) as tc:
        import os
        _bufs = dict(x.split("=") for x in os.environ.get("KBUFS", "").split(",") if x)
        bf = lambda k, d: int(_bufs.get(k, d))
        with (
            tc.tile_pool(name="const", bufs=1) as const,
            tc.tile_pool(name="xtp", bufs=bf("xt", 2)) as xtp,
            tc.tile_pool(name="rowp", bufs=bf("row", 2)) as rowp,
            tc.tile_pool(name="bigp", bufs=bf("big", 2)) as bigp,
            tc.tile_pool(name="innp", bufs=bf("inn", 8)) as innp,
            tc.tile_pool(name="yp", bufs=bf("yp", 3)) as yp,
            tc.tile_pool(name="rows_ps", bufs=bf("rps", 1), space="PSUM") as rows_ps,
            tc.tile_pool(name="v_ps", bufs=bf("vps", 2), space="PSUM") as v_ps,
            tc.tile_pool(name="bc_ps", bufs=bf("bcps", 1), space="PSUM") as bc_ps,
            tc.tile_pool(name="y_ps", bufs=bf("yps", 2), space="PSUM") as y_ps,
        ):
            # ---- resident weights, ordered by first use so PE starts ASAP:
            # kp8/kp4 + chunk-0 xt + basis first, then vw, sel; owt is
            # deferred until after the first xt prefetch (needed ~30us in).
            kp8 = const.tile([128, KB, 72], F32R, name="kp8", tag="kp8")
            nc.sync.dma_start(
                out=kp8,
                in_=kpack[0:W, :].rearrange("(kb p) m -> p kb m", p=128))
            kp4 = const.tile([4, 72], F32R, name="kpbas", tag="kpbas")
            nc.sync.dma_start(out=kp4, in_=kpack[W:W + 4, :])

            def load_xt(ci, split=False):
                c0 = ci * CHUNK
                ts = []
                for kb in range(KB):
                    t = xtp.tile([128, CHUNK], F32R, name=f"xt{kb}",
                                 tag=f"xt{kb}")
                    eng = nc.scalar if (split and kb % 2) else nc.sync
                    eng.dma_start(
                        out=t, in_=xtb[kb * 128:(kb + 1) * 128, c0:c0 + CHUNK])
                    ts.append(t)
                bas = xtp.tile([4, CHUNK], F32R, name="bas", tag="bas")
                (nc.scalar if split else nc.sync).dma_start(
                    out=bas, in_=xtb[W:W + 4, c0:c0 + CHUNK])
                return ts, bas

            xt_cur = load_xt(0, split=True)

            # weights on the ACT HW-DGE ring, parallel with xt0 on SP
            vw_sb = []
            for p in range(NPAIR):
                t = const.tile([128, KB, 128], F32R, name=f"vw{p}", tag=f"vw{p}")
                nc.scalar.dma_start(
                    out=t,
                    in_=vwp[p].rearrange("(kb q) m -> q kb m", q=128))
                vw_sb.append([t[:, kb, :] for kb in range(KB)])

            sel8 = const.tile([72, NPAIR, 128], F32R, name="sel8", tag="sel8")
            nc.scalar.dma_start(out=sel8, in_=selp.rearrange("p r m -> r p m"))
            sel_sb = [sel8[0:8, p, :] for p in range(NPAIR)]

            owt_sb = []

            def load_owt():
                for p in range(NPAIR):
                    t = const.tile([128, W], F32R, name=f"owt{p}",
                                   tag=f"owt{p}")
                    nc.scalar.dma_start(out=t, in_=owtp[p, :, :])
                    owt_sb.append(t)

            ones8 = const.tile([HL, CHUNK], F32)
            nc.vector.memset(ones8, 1.0)

            # ---- per-chunk pipeline state ----
            z_prev = [None] * NPAIR     # z carry tiles per pair
            c_prev = None               # C-scan carry tile
            den_prev = None             # previous chunk's den tile
            pend = None                 # (inner tiles, c0) awaiting stage-3

            total = NCHUNK * reps
            cis = [i % NCHUNK for i in range(total)]

            def emit_back(inner, c0):
                """Stage 3 for a completed chunk: y matmuls + copies + DMA."""
                for nb in range(NBLK):
                    for wh in range(2):
                        yps = y_ps.tile([128, 512], F32, tag="y")
                        for p in range(NPAIR):
                            nc.tensor.matmul(
                                yps,
                                lhsT=inner[p][:, nb * 128:(nb + 1) * 128],
                                rhs=owt_sb[p][:, wh * 512:(wh + 1) * 512],
                                start=(p == 0), stop=(p == NPAIR - 1))
                        y_sb = yp.tile([128, 512], F32, tag="y_sb")
                        nc.scalar.copy(y_sb, yps)
                        nc.sync.dma_start(
                            out=y[c0 + nb * 128:c0 + (nb + 1) * 128,
                                  wh * 512:(wh + 1) * 512],
                            in_=y_sb)

            for it in range(total):
                ci = cis[it]
                c0 = ci * CHUNK
                first = (it == 0)

                xt, bas = xt_cur
                if it + 1 < total:
                    xt_next = load_xt(cis[it + 1])
                else:
                    xt_next = None

                # ---- row projections: [24, CHUNK] psum ----
                rows = rows_ps.tile([72, CHUNK], F32, tag="rows")
                for kb in range(KB):
                    nc.tensor.matmul(rows, lhsT=kp8[:, kb, :], rhs=xt[kb],
                                     start=(kb == 0), stop=False)
                nc.tensor.matmul(rows, lhsT=kp4, rhs=bas,
                                 start=False, stop=True)

                # exps: c, e, g  [8, CHUNK] each
                c_t = rowp.tile([HL, CHUNK], F32R, tag="c_t")
                nc.scalar.activation(c_t, rows[0:8, :], AF.Exp)
                e_t = rowp.tile([HL, CHUNK], F32R, tag="e_t")
                nc.scalar.activation(e_t, rows[32:40, :], AF.Exp)
                if mode == "rt":
                    g72 = rowp.tile([72, CHUNK], F32R, tag="g72")
                    nc.scalar.activation(g72[64:72, :], rows[64:72, :], AF.Exp)
                else:
                    g_t = rowp.tile([HL, CHUNK], F32R, tag="g_t")
                    nc.scalar.activation(g_t, rows[64:72, :], AF.Exp)

                # C = cumsum(c) chunk-chained (Pool)
                c_ch = rowp.tile([HL, CHUNK], F32, tag="c_ch")
                nc.vector.tensor_tensor_scan(
                    c_ch, data0=ones8, data1=c_t,
                    initial=(0.0 if c_prev is None
                             else c_prev[:, CHUNK - 1:CHUNK]),
                    op0=OP.mult, op1=OP.add)
                c_prev = c_ch
                # den = C + e ; rden = 1/den ; w = c*rden (DVE)
                den = rowp.tile([HL, CHUNK], F32, tag="den")
                nc.vector.tensor_add(den, c_ch, e_t)
                rden = rowp.tile([HL, CHUNK], F32, tag="rden")
                nc.vector.reciprocal_approx_fast(out=rden, in_=den)
                w_t = rowp.tile([HL, CHUNK], F32R, tag="w_t")
                nc.vector.tensor_mul(w_t, c_t, rden)
                if mode == "rt":
                    w40 = rowp.tile([40, CHUNK], F32R, tag="w40")
                    nc.scalar.copy(w40[32:40, :], w_t)
                # ratio'[t] = den[t-1] * rden[t]
                rat = rowp.tile([HL, CHUNK], F32R, tag="rat")
                nc.vector.tensor_mul(rat[:, 1:CHUNK], den[:, 0:CHUNK - 1],
                                     rden[:, 1:CHUNK])
                if den_prev is None:
                    # any finite value works: initial z state is 0
                    nc.vector.tensor_copy(rat[:, 0:1], ones8[:, 0:1])
                else:
                    nc.vector.tensor_mul(rat[:, 0:1],
                                         den_prev[:, CHUNK - 1:CHUNK],
                                         rden[:, 0:1])
                den_prev = den

                # ---- values + broadcasts, interleaved on PE ----
                vps_l = [None] * NPAIR
                bc_l = [None] * NPAIR

                def emit_values(p):
                    vps = v_ps.tile([128, CHUNK], F32, tag="v")
                    for kb in range(KB):
                        nc.tensor.matmul(vps, lhsT=vw_sb[p][kb], rhs=xt[kb],
                                         start=(kb == 0), stop=(kb == KB - 1))
                    vps_l[p] = vps
                    v_sb = bigp.tile([128, CHUNK], F32, tag="v_sb")
                    nc.scalar.copy(v_sb, vps)
                    return v_sb

                def emit_bcast(p):
                    r_rep = bc_ps.tile([128, CHUNK], F32, tag="r_rep")
                    w_rep = bc_ps.tile([128, CHUNK], F32, tag="w_rep")
                    g_rep = bc_ps.tile([128, CHUNK], F32, tag="g_rep")
                    if mode == "rt":
                        # three row-groups (0/32/64) -> concurrent PE tiles
                        nc.tensor.matmul(r_rep, lhsT=sel8[0:8, p, :], rhs=rat,
                                         start=True, stop=True)
                        nc.tensor.matmul(w_rep, lhsT=sel8[32:40, p, :],
                                         rhs=w40[32:40, :],
                                         start=True, stop=True)
                        nc.tensor.matmul(g_rep, lhsT=sel8[64:72, p, :],
                                         rhs=g72[64:72, :],
                                         start=True, stop=True)
                    else:
                        nc.tensor.matmul(r_rep, lhsT=sel_sb[p], rhs=rat,
                                         start=True, stop=True)
                        nc.tensor.matmul(w_rep, lhsT=sel_sb[p], rhs=w_t,
                                         start=True, stop=True)
                        nc.tensor.matmul(g_rep, lhsT=sel_sb[p], rhs=g_t,
                                         start=True, stop=True)
                    w_sb = bigp.tile([128, CHUNK], F32, tag="w_sb")
                    nc.vector.tensor_copy(w_sb, w_rep)
                    bc_l[p] = (r_rep, g_rep, w_sb)

                v_sb_l = [None] * NPAIR
                # PE order: v0 v1 v2 bc0 v3 bc1 bc2 bc3
                v_sb_l[0] = emit_values(0)
                v_sb_l[1] = emit_values(1)
                v_sb_l[2] = emit_values(2)
                emit_bcast(0)
                v_sb_l[3] = emit_values(3)
                emit_bcast(1)
                emit_bcast(2)
                emit_bcast(3)

                # ---- backend: vw, scan, t2, inner per pair ----
                inner = []
                for p in range(NPAIR):
                    r_rep, g_rep, w_sb = bc_l[p]
                    vw = bigp.tile([128, CHUNK], F32, tag="vw", bufs=4)
                    nc.gpsimd.tensor_mul(vw, v_sb_l[p], w_sb)
                    z_sb = bigp.tile([128, CHUNK], F32, tag="z_sb", bufs=8)
                    nc.vector.tensor_tensor_scan(
                        z_sb, data0=r_rep, data1=vw,
                        initial=(0.0 if z_prev[p] is None
                                 else z_prev[p][:, CHUNK - 1:CHUNK]),
                        op0=OP.mult, op1=OP.add)
                    z_prev[p] = z_sb
                    t2 = bigp.tile([128, CHUNK], F32, tag="t2")
                    nc.vector.tensor_mul(t2, vw, g_rep)
                    inn = innp.tile([128, CHUNK], F32R, name="inner",
                                    tag="inner")
                    nc.gpsimd.tensor_add(inn, z_sb, t2)
                    inner.append(inn)

                if it == 0:
                    load_owt()

                # ---- lagged stage 3 ----
                if pend is not None:
                    emit_back(*pend)
                pend = (inner, c0)
                xt_cur = xt_next

            emit_back(*pend)

    nc.finalize()
    _NC_CACHE[key] = nc
    return nc


def _host_prep(x, k1, k2, k3, a1, a2, b1, b2, c, value_weight, output_weight):
    """Build the 8 per-core input maps."""
    x = np.asarray(x, np.float32)
    k1 = np.asarray(k1, np.float32)
    k2 = np.asarray(k2, np.float32)
    k3 = np.asarray(k3, np.float32)
    a1 = np.asarray(a1, np.float64)[..., 0]   # [H, P]
    a2 = np.asarray(a2, np.float64)[..., 0]
    b1 = np.asarray(b1, np.float64)[..., 0]
    b2 = np.asarray(b2, np.float64)[..., 0]
    cc = np.asarray(c, np.float64)[..., 0]
    vw = np.asarray(value_weight, np.float32)   # [H, W, A]
    ow = np.asarray(output_weight, np.float32)  # [H, W, A]

    n = np.linspace(0.0, 1.0, N)
    basis = np.stack([np.ones_like(n), n, n * n, n ** 3]).astype(np.float32)

    def taylor(a, b):
        # coef[k, h] of n^k for sum_p c*sin(a*n+b)
        s, co = np.sin(b), np.cos(b)
        c0 = (cc * s).sum(1)
        c1 = (cc * a * co).sum(1)
        c2 = -(cc * a * a * s).sum(1) / 2.0
        c3 = -(cc * a ** 3 * co).sum(1) / 6.0
        return np.stack([c0, c1, c2, c3])      # [4, H]

    p1c = taylor(a1, b1)
    p2c = taylor(a2, b2)

    xt_by_b = [np.empty((W + 4, N), np.float32) for _ in range(B)]
    for b in range(B):
        xt_by_b[b][:W] = x[b].T
        xt_by_b[b][W:] = basis

    selp = np.zeros((NPAIR, 72, 128), np.float32)
    for p in range(NPAIR):
        for base in (0, 32, 64):
            selp[p, base + 2 * p, 0:64] = 1.0
            selp[p, base + 2 * p + 1, 64:128] = 1.0

    in_maps = []
    for core in range(8):
        b, half = divmod(core, 2)
        hs = slice(half * HL, (half + 1) * HL)
        kpk = np.zeros((W + 4, 72), np.float32)
        # zc -> c = exp(x@k1 + p1)  (row groups 32-aligned: PSUM reads
        # by the ACT engine require 32-aligned partition bases)
        kpk[:W, 0:8] = k1[hs].T
        kpk[W:, 0:8] = p1c[:, hs]
        # ze -> e = exp(x@(k2-k3) - p2)
        kpk[:W, 32:40] = (k2[hs] - k3[hs]).T
        kpk[W:, 32:40] = -p2c[:, hs]
        # zg -> g = e/c = exp(x@(k2-k3-k1) - p2 - p1)
        kpk[:W, 64:72] = (k2[hs] - k3[hs] - k1[hs]).T
        kpk[W:, 64:72] = -(p2c[:, hs] + p1c[:, hs])

        vwp = np.empty((NPAIR, W, 128), np.float32)
        owtp = np.empty((NPAIR, 128, W), np.float32)
        for p in range(NPAIR):
            h0 = half * HL + 2 * p
            vwp[p, :, 0:64] = vw[h0]
            vwp[p, :, 64:128] = vw[h0 + 1]
            owtp[p, 0:64, :] = ow[h0].T
            owtp[p, 64:128, :] = ow[h0 + 1].T

        in_maps.append(dict(xtb=xt_by_b[b], kpack=kpk, vwp=vwp, owtp=owtp,
                            selp=selp))
    return in_maps


LAST_RESULT = None


def kernel(**inputs) -> np.ndarray:
    global LAST_RESULT
    in_maps = _host_prep(**inputs)
    nc = _build()
    res = None
    for attempt in range(3):
        try:
            res = run_bass_kernel_spmd(nc, in_maps, core_ids=list(range(8)))
            break
        except Exception:
            # transient axon-tunnel / device flakes happen; retry
            if attempt == 2:
                raise
            import time
            time.sleep(5)
    LAST_RESULT = res
    out = np.empty((B, N, W), np.float32)
    for b in range(B):
        out[b] = res.results[2 * b]["y"] + res.results[2 * b + 1]["y"]
    return out
